# revision 10
# baseline (speedup 1.0000x reference)
"""Dilated attention (LongNet-style) Bass kernel for 8 Trainium2 NeuronCores.

Problem: q,k,v of shape (B=2, S=8192, H=16, D=64) fp32.
4 head-groups x (segment length s, dilation r) with s/r == 1024 for every
group, so the whole computation is 120 identical 1024x1024x64 attention
sub-problems plus a per-(batch, head, channel) sum-normalization.

Sharding: core = b*4 + j owns heads {j, 4+j, 8+j, 12+j} of batch b, i.e.
one head from each group -> 8+4+2+1 = 15 sub-problems per core (perfectly
balanced), and every (batch, head) lives on exactly one core.

The device computes only the O(S^2) part (scores, exp, PV); the O(S)
normalization (1/l, hi+lo fold, channel sums, final scale) runs on the
host in fp64, which is both faster (no cross-partition folds on device —
DVE lanes are partition-locked and sub-32-partition ops at base 64
produce garbage on HW) and more accurate.

Numerics: the final x / sum(x) normalization is badly conditioned; error
sources that are correlated across queries (K and V quantization) are
amplified ~30-100x, so K and V are fed as bf16-hi + lo pairs (~22
effective bits) while Q and E ride single fp32r (~13 effective bits,
uncorrelated errors average out). All matmuls run in fp32r, which
streams at full bf16 rate (1 cycle/row for N>=256).

Per sub-problem on-device (fp32 PSUM accumulation):
  S^T[k,q] = [khi;klo].T @ [q;q]       one stacked K=128 fp32r matmul per
                                       512-col half per key chunk
  E        = exp(S^T), ACT writes fp32r directly
  PV       = [Vhi|1|Vlo].T @ E         M=128-packed: rows 0:64 hi-part,
                                       row 64 = softmax denom l,
                                       rows 65:128 lo-part
  out      = PV copied to SBUF, DMA'd raw; host folds + normalizes.
"""

import os
import numpy as np
import ml_dtypes

import concourse.bass as bass
import concourse.bacc as bacc
import concourse.mybir as mybir
import concourse.tile as tile
from concourse import bass_utils

# ---------------------------------------------------------------- constants
B, S, H, D = 2, 8192, 16, 64
SEGMENT_LENGTHS = [1024, 2048, 4096, 8192]
DILATION_RATES = [1, 2, 4, 8]
NUM_GROUPS = 4
GROUP_HEADS = H // NUM_GROUPS  # 4
SEGS_PER_GROUP = [S // s for s in SEGMENT_LENGTHS]  # [8, 4, 2, 1]
NPROB = sum(SEGS_PER_GROUP)  # 15 problems per core
SL = 1024          # per-problem sequence length (s // r, same for all groups)
NCHUNK = SL // 128  # 8 key chunks
N_CORES = 8
SCALE = 1.0 / np.sqrt(D)

FP32 = mybir.dt.float32
FP32R = mybir.dt.float32r
VW = D + 1  # 65: hi rows plus the l row


def _problem_list(j):
    """15 (group, head, seg) tuples for local head-slot j, head-contiguous."""
    out = []
    for g in range(NUM_GROUPS):
        head = g * GROUP_HEADS + j
        for seg in range(SEGS_PER_GROUP[g]):
            out.append((g, head, seg))
    return out


def _positions(g, seg):
    s, r = SEGMENT_LENGTHS[g], DILATION_RATES[g]
    offset = g % r
    return seg * s + offset + r * np.arange(SL)


# ---------------------------------------------------------------- device IR
def _build_tile_program(ctx, tc, out_ap, qd_ap, kd_ap, vp_ap):
    nc = tc.nc
    EXP = mybir.ActivationFunctionType.Exp

    qk_pool = ctx.enter_context(tc.tile_pool(name="qk", bufs=3))
    k_pool = ctx.enter_context(tc.tile_pool(name="kt", bufs=3))
    vp_pool = ctx.enter_context(tc.tile_pool(name="vp", bufs=3))
    e_pool = ctx.enter_context(tc.tile_pool(name="exps", bufs=3))
    pvsb_pool = ctx.enter_context(tc.tile_pool(name="pvsb", bufs=3))
    spsum = ctx.enter_context(tc.tile_pool(name="spsum", bufs=2, space="PSUM"))
    pvpsum = ctx.enter_context(tc.tile_pool(name="pvpsum", bufs=2, space="PSUM"))

    reps = int(os.environ.get("DILATED_REPS", "1"))
    for p in [i % NPROB for i in range(reps * NPROB)]:
        # spread input DMAs over two queues (SP, GpSimd) so the first
        # problem's data lands sooner; ACT stays DMA-free
        k_t = k_pool.tile([128, SL], FP32R)
        nc.gpsimd.dma_start(out=k_t, in_=kd_ap[p])
        qk_t = qk_pool.tile([128, SL], FP32R)
        nc.sync.dma_start(out=qk_t[0:D, :], in_=qd_ap[p])
        nc.sync.dma_start(out=qk_t[D:128, :], in_=qd_ap[p])
        vp_t = vp_pool.tile([128, NCHUNK * 128], FP32R)
        nc.gpsimd.dma_start(out=vp_t, in_=vp_ap[p])

        pv_ps = None
        for c in range(NCHUNK):
            # S^T chunk c = khi.T q + klo.T q, one stacked K=128 matmul/half
            s_ps = spsum.tile([128, SL], FP32, tag="s")
            for h in range(2):
                hs = slice(h * 512, (h + 1) * 512)
                nc.tensor.matmul(
                    out=s_ps[:, hs],
                    lhsT=k_t[:, c * 128:(c + 1) * 128],
                    rhs=qk_t[:, hs],
                    start=True, stop=True,
                )

            e_t = e_pool.tile([128, SL], FP32R)
            nc.scalar.activation(out=e_t, in_=s_ps, func=EXP)

            if pv_ps is None:
                pv_ps = pvpsum.tile([128, SL], FP32, tag="pv")
            for h in range(2):
                hs = slice(h * 512, (h + 1) * 512)
                nc.tensor.matmul(      # [Vhi | 1 | Vlo].T @ E
                    out=pv_ps[:, hs],
                    lhsT=vp_t[:, c * 128:(c + 1) * 128],
                    rhs=e_t[:, hs],
                    start=(c == 0), stop=(c == NCHUNK - 1),
                )

        # evacuate PV psum to SBUF and ship it; host does the rest.
        # Split by column half: bank-level deps let the h0 copy overlap the
        # final h1 matmul, shrinking the kernel tail.
        pv_sb = pvsb_pool.tile([128, SL], FP32)
        for h in range(2):
            hs = slice(h * 512, (h + 1) * 512)
            nc.vector.tensor_copy(out=pv_sb[:, hs], in_=pv_ps[:, hs])
            nc.gpsimd.dma_start(out=out_ap[p][:, hs], in_=pv_sb[:, hs])


# Cache: the Bass program is identical for every call (and every core).
_CACHED = {}


def _get_program():
    key = os.environ.get("DILATED_REPS", "1")
    if key in _CACHED:
        return _CACHED[key]
    nc = bacc.Bacc("TRN2", target_bir_lowering=False, debug=False)
    qd = nc.dram_tensor("qd", [NPROB, D, SL], FP32R,
                        kind="ExternalInput").ap()
    kd = nc.dram_tensor("kd", [NPROB, 128, SL], FP32R,
                        kind="ExternalInput").ap()
    vp = nc.dram_tensor("vp", [NPROB, 128, NCHUNK * 128], FP32R,
                        kind="ExternalInput").ap()
    out = nc.dram_tensor("out", [NPROB, 128, SL], FP32,
                         kind="ExternalOutput").ap()
    from contextlib import ExitStack
    with tile.TileContext(nc) as tc, ExitStack() as ctx:
        _build_tile_program(ctx, tc, out, qd, kd, vp)
    nc.compile()
    _CACHED[key] = nc
    return nc


# ---------------------------------------------------------------- host glue
def _bf16hi(x):
    return x.astype(ml_dtypes.bfloat16).astype(np.float32)


def _prep_core(q, k, v, b, j):
    """Build the qd/kd/vp device inputs for core (b, j). q is pre-scaled."""
    qd = np.empty((NPROB, D, SL), dtype=np.float32)
    kd = np.empty((NPROB, 128, SL), dtype=np.float32)
    vp = np.empty((NPROB, 128, NCHUNK * 128), dtype=np.float32)
    ones = np.ones((SL, 1), np.float32)
    for p, (g, head, seg) in enumerate(_problem_list(j)):
        pos = _positions(g, seg)
        qd[p] = q[b, pos, head, :].T  # [64, 1024] fp32, already scaled
        kT = k[b, pos, head, :].T
        khi = _bf16hi(kT)
        kd[p, 0:D] = khi
        kd[p, D:128] = kT - khi
        vs = v[b, pos, head, :]  # [1024, 64] fp32
        vhi = _bf16hi(vs)
        vlo = vs - vhi
        block = np.concatenate([vhi, ones, vlo[:, 0:D - 1]], axis=1)
        vp[p] = (block.reshape(NCHUNK, 128, 128)
                 .transpose(1, 0, 2).reshape(128, NCHUNK * 128))
    return {"qd": qd, "kd": kd, "vp": vp}


def _postprocess_core(dev_out):
    """[NPROB, 128, SL] raw PV -> [NPROB, D, SL] normalized attention out.

    Folds the packed lo rows into the hi rows, divides by the softmax
    denominator row, then applies the per-(head, channel) 1/(4*sum)
    normalization across each head's segments. fp64 throughout.
    """
    pv = dev_out.astype(np.float64)
    x = pv[:, 0:D, :].copy()
    x[:, 0:D - 1, :] += pv[:, VW:128, :]
    x /= pv[:, D:VW, :]  # divide by l row
    outp = np.empty((NPROB, D, SL), dtype=np.float32)
    i = 0
    for nseg in SEGS_PER_GROUP:
        xs = x[i:i + nseg]  # [nseg, D, SL]
        hsum = xs.sum(axis=(0, 2), keepdims=True)  # [1, D, 1]
        outp[i:i + nseg] = (xs / (NUM_GROUPS * hsum)).astype(np.float32)
        i += nseg
    return outp


def kernel(query, key, value, _run_kw=None):
    q = np.asarray(query, dtype=np.float32)
    k = np.asarray(key, dtype=np.float32)
    v = np.asarray(value, dtype=np.float32)
    qs = q * SCALE  # fold softmax scale into q

    nc = _get_program()
    in_maps = []
    core_meta = []
    for core in range(N_CORES):
        b, j = divmod(core, NUM_GROUPS)
        in_maps.append(_prep_core(qs, k, v, b, j))
        core_meta.append((b, j))

    kw = dict(_run_kw or {})
    kw.pop("result", None)
    res = bass_utils.run_bass_kernel_spmd(
        nc, in_maps, core_ids=list(range(N_CORES)), **kw)

    out = np.zeros((B, S, H, D), dtype=np.float32)
    for core in range(N_CORES):
        b, j = core_meta[core]
        dev_out = _postprocess_core(res.results[core]["out"])
        for p, (g, head, seg) in enumerate(_problem_list(j)):
            pos = _positions(g, seg)
            out[b, pos, head, :] = dev_out[p].T
    if _run_kw is not None:
        _run_kw["result"] = res
    return out


# revision 12
# speedup vs baseline: 1.1793x; 1.1793x over previous
"""Dilated attention (LongNet-style) Bass kernel for 8 Trainium2 NeuronCores.

Problem: q,k,v of shape (B=2, S=8192, H=16, D=64) fp32.
4 head-groups x (segment length s, dilation r) with s/r == 1024 for every
group, so the whole computation is 120 identical 1024x1024x64 attention
sub-problems plus a per-(batch, head, channel) sum-normalization.

Sharding: core = b*4 + j owns heads {j, 4+j, 8+j, 12+j} of batch b, i.e.
one head from each group -> 8+4+2+1 = 15 sub-problems per core (perfectly
balanced), and every (batch, head) lives on exactly one core.

The device computes only the O(S^2) part (scores, exp, PV); the O(S)
normalization (1/l, hi+lo fold, channel sums, final scale) runs on the
host in fp64, which is both faster (no cross-partition folds on device —
DVE lanes are partition-locked and sub-32-partition ops at base 64
produce garbage on HW) and more accurate.

Numerics: the final x / sum(x) normalization is badly conditioned; error
sources that are correlated across queries (K and V quantization) are
amplified ~30-100x, so K and V are fed as bf16-hi + lo pairs (~22
effective bits) while Q and E ride single fp32r (~13 effective bits,
uncorrelated errors average out). All matmuls run in fp32r, which
streams at full bf16 rate (1 cycle/row for N>=256).

Per sub-problem on-device (fp32 PSUM accumulation):
  S^T[k,q] = [khi;klo].T @ [q;q]       one stacked K=128 fp32r matmul per
                                       512-col half per key chunk
  E        = exp(S^T), ACT writes fp32r directly
  PV       = [Vhi|1|Vlo].T @ E         M=128-packed: rows 0:64 hi-part,
                                       row 64 = softmax denom l,
                                       rows 65:128 lo-part
  out      = PV copied to SBUF, DMA'd raw; host folds + normalizes.
"""

import os
import numpy as np
import ml_dtypes

import concourse.bass as bass
import concourse.bacc as bacc
import concourse.mybir as mybir
import concourse.tile as tile
from concourse import bass_utils

# ---------------------------------------------------------------- constants
B, S, H, D = 2, 8192, 16, 64
SEGMENT_LENGTHS = [1024, 2048, 4096, 8192]
DILATION_RATES = [1, 2, 4, 8]
NUM_GROUPS = 4
GROUP_HEADS = H // NUM_GROUPS  # 4
SEGS_PER_GROUP = [S // s for s in SEGMENT_LENGTHS]  # [8, 4, 2, 1]
NPROB = sum(SEGS_PER_GROUP)  # 15 problems per core
SL = 1024          # per-problem sequence length (s // r, same for all groups)
NCHUNK = SL // 128  # 8 key chunks
N_CORES = 8
SCALE = 1.0 / np.sqrt(D)

FP32 = mybir.dt.float32
FP32R = mybir.dt.float32r
VW = D + 1  # 65: hi rows plus the l row


def _problem_list(j):
    """15 (group, head, seg) tuples for local head-slot j, head-contiguous."""
    out = []
    for g in range(NUM_GROUPS):
        head = g * GROUP_HEADS + j
        for seg in range(SEGS_PER_GROUP[g]):
            out.append((g, head, seg))
    return out


def _positions(g, seg):
    s, r = SEGMENT_LENGTHS[g], DILATION_RATES[g]
    offset = g % r
    return seg * s + offset + r * np.arange(SL)


# ---------------------------------------------------------------- device IR
def _build_tile_program(ctx, tc, out_ap, qd_ap, kd_ap, vp_ap):
    nc = tc.nc
    EXP = mybir.ActivationFunctionType.Exp

    qk_pool = ctx.enter_context(tc.tile_pool(name="qk", bufs=3))
    k_pool = ctx.enter_context(tc.tile_pool(name="kt", bufs=3))
    vp_pool = ctx.enter_context(tc.tile_pool(name="vp", bufs=3))
    e_pool = ctx.enter_context(tc.tile_pool(name="exps", bufs=3))
    pvsb_pool = ctx.enter_context(tc.tile_pool(name="pvsb", bufs=3))
    scr_pool = ctx.enter_context(tc.tile_pool(name="scr", bufs=1))
    spsum = ctx.enter_context(tc.tile_pool(name="spsum", bufs=3, space="PSUM"))
    pvpsum = ctx.enter_context(tc.tile_pool(name="pvpsum", bufs=1, space="PSUM"))

    # preload the exp table during the initial DMA fill: a dummy
    # activation on a memset scratch tile pulls the 1.3us ACT_TABLE_LOAD
    # off the critical path
    warm = scr_pool.tile([1, 2], FP32, tag="warm")
    nc.gpsimd.memset(warm, 0.0)
    warm_o = scr_pool.tile([1, 2], FP32, tag="warmo")
    nc.scalar.activation(out=warm_o, in_=warm, func=EXP)

    reps = int(os.environ.get("DILATED_REPS", "1"))
    plist = [i % NPROB for i in range(reps * NPROB)]
    for i, p in enumerate(plist):
        last = i == len(plist) - 1
        # steady-state inputs ride the SP queue; the first problem's k/vp
        # go on the idle GpSimd queue in parallel to cut the pipeline fill
        k_t = k_pool.tile([128, SL], FP32R)
        (nc.gpsimd if i == 0 else nc.sync).dma_start(out=k_t, in_=kd_ap[p])
        qk_t = qk_pool.tile([128, SL], FP32R)
        nc.sync.dma_start(out=qk_t[0:D, :], in_=qd_ap[p])
        nc.sync.dma_start(out=qk_t[D:128, :], in_=qd_ap[p])
        vp_t = vp_pool.tile([128, NCHUNK * 128], FP32R)
        (nc.gpsimd if i == 0 else nc.sync).dma_start(out=vp_t, in_=vp_ap[p])

        pv_ps = None
        for c in range(NCHUNK):
            # S^T chunk c = khi.T q + klo.T q, one stacked K=128 matmul/half
            s_ps = spsum.tile([128, SL], FP32, tag="s")
            for h in range(2):
                hs = slice(h * 512, (h + 1) * 512)
                nc.tensor.matmul(
                    out=s_ps[:, hs],
                    lhsT=k_t[:, c * 128:(c + 1) * 128],
                    rhs=qk_t[:, hs],
                    start=True, stop=True,
                )

            e_t = e_pool.tile([128, SL], FP32R)
            nc.scalar.activation(out=e_t, in_=s_ps, func=EXP)

            if pv_ps is None:
                pv_ps = pvpsum.tile([128, SL], FP32, tag="pv")
            for h in range(2):
                hs = slice(h * 512, (h + 1) * 512)
                nc.tensor.matmul(      # [Vhi | 1 | Vlo].T @ E
                    out=pv_ps[:, hs],
                    lhsT=vp_t[:, c * 128:(c + 1) * 128],
                    rhs=e_t[:, hs],
                    start=(c == 0), stop=(c == NCHUNK - 1),
                )

        # evacuate PV psum to SBUF and ship it; host does the rest.
        # The last problem's output is split across two DMA queues so the
        # kernel tail is half a transfer, not a full one.
        pv_sb = pvsb_pool.tile([128, SL], FP32)
        nc.vector.tensor_copy(out=pv_sb, in_=pv_ps)
        if last:
            nc.gpsimd.dma_start(out=out_ap[p][:, 0:512], in_=pv_sb[:, 0:512])
            nc.sync.dma_start(out=out_ap[p][:, 512:SL], in_=pv_sb[:, 512:SL])
        else:
            nc.gpsimd.dma_start(out=out_ap[p], in_=pv_sb)


# Cache: the Bass program is identical for every call (and every core).
_CACHED = {}


def _get_program():
    key = os.environ.get("DILATED_REPS", "1")
    if key in _CACHED:
        return _CACHED[key]
    nc = bacc.Bacc("TRN2", target_bir_lowering=False, debug=False)
    qd = nc.dram_tensor("qd", [NPROB, D, SL], FP32R,
                        kind="ExternalInput").ap()
    kd = nc.dram_tensor("kd", [NPROB, 128, SL], FP32R,
                        kind="ExternalInput").ap()
    vp = nc.dram_tensor("vp", [NPROB, 128, NCHUNK * 128], FP32R,
                        kind="ExternalInput").ap()
    out = nc.dram_tensor("out", [NPROB, 128, SL], FP32,
                         kind="ExternalOutput").ap()
    from contextlib import ExitStack
    with tile.TileContext(nc) as tc, ExitStack() as ctx:
        _build_tile_program(ctx, tc, out, qd, kd, vp)
    nc.compile()
    _CACHED[key] = nc
    return nc


# ---------------------------------------------------------------- host glue
def _bf16hi(x):
    return x.astype(ml_dtypes.bfloat16).astype(np.float32)


def _prep_core(q, k, v, b, j):
    """Build the qd/kd/vp device inputs for core (b, j). q is pre-scaled."""
    qd = np.empty((NPROB, D, SL), dtype=np.float32)
    kd = np.empty((NPROB, 128, SL), dtype=np.float32)
    vp = np.empty((NPROB, 128, NCHUNK * 128), dtype=np.float32)
    ones = np.ones((SL, 1), np.float32)
    for p, (g, head, seg) in enumerate(_problem_list(j)):
        pos = _positions(g, seg)
        qd[p] = q[b, pos, head, :].T  # [64, 1024] fp32, already scaled
        kT = k[b, pos, head, :].T
        khi = _bf16hi(kT)
        kd[p, 0:D] = khi
        kd[p, D:128] = kT - khi
        vs = v[b, pos, head, :]  # [1024, 64] fp32
        vhi = _bf16hi(vs)
        vlo = vs - vhi
        block = np.concatenate([vhi, ones, vlo[:, 0:D - 1]], axis=1)
        vp[p] = (block.reshape(NCHUNK, 128, 128)
                 .transpose(1, 0, 2).reshape(128, NCHUNK * 128))
    return {"qd": qd, "kd": kd, "vp": vp}


def _postprocess_core(dev_out):
    """[NPROB, 128, SL] raw PV -> [NPROB, D, SL] normalized attention out.

    Folds the packed lo rows into the hi rows, divides by the softmax
    denominator row, then applies the per-(head, channel) 1/(4*sum)
    normalization across each head's segments. fp64 throughout.
    """
    pv = dev_out.astype(np.float64)
    x = pv[:, 0:D, :].copy()
    x[:, 0:D - 1, :] += pv[:, VW:128, :]
    x /= pv[:, D:VW, :]  # divide by l row
    outp = np.empty((NPROB, D, SL), dtype=np.float32)
    i = 0
    for nseg in SEGS_PER_GROUP:
        xs = x[i:i + nseg]  # [nseg, D, SL]
        hsum = xs.sum(axis=(0, 2), keepdims=True)  # [1, D, 1]
        outp[i:i + nseg] = (xs / (NUM_GROUPS * hsum)).astype(np.float32)
        i += nseg
    return outp


def kernel(query, key, value, _run_kw=None):
    q = np.asarray(query, dtype=np.float32)
    k = np.asarray(key, dtype=np.float32)
    v = np.asarray(value, dtype=np.float32)
    qs = q * SCALE  # fold softmax scale into q

    nc = _get_program()
    in_maps = []
    core_meta = []
    for core in range(N_CORES):
        b, j = divmod(core, NUM_GROUPS)
        in_maps.append(_prep_core(qs, k, v, b, j))
        core_meta.append((b, j))

    kw = dict(_run_kw or {})
    kw.pop("result", None)
    res = bass_utils.run_bass_kernel_spmd(
        nc, in_maps, core_ids=list(range(N_CORES)), **kw)

    out = np.zeros((B, S, H, D), dtype=np.float32)
    for core in range(N_CORES):
        b, j = core_meta[core]
        dev_out = _postprocess_core(res.results[core]["out"])
        for p, (g, head, seg) in enumerate(_problem_list(j)):
            pos = _positions(g, seg)
            out[b, pos, head, :] = dev_out[p].T
    if _run_kw is not None:
        _run_kw["result"] = res
    return out


# revision 14
# speedup vs baseline: 1.2115x; 1.0272x over previous
"""Dilated attention (LongNet-style) Bass kernel for 8 Trainium2 NeuronCores.

Problem: q,k,v of shape (B=2, S=8192, H=16, D=64) fp32.
4 head-groups x (segment length s, dilation r) with s/r == 1024 for every
group, so the whole computation is 120 identical 1024x1024x64 attention
sub-problems plus a per-(batch, head, channel) sum-normalization.

Sharding: core = b*4 + j owns heads {j, 4+j, 8+j, 12+j} of batch b, i.e.
one head from each group -> 8+4+2+1 = 15 sub-problems per core (perfectly
balanced), and every (batch, head) lives on exactly one core.

The device computes only the O(S^2) part (scores, exp, PV); the O(S)
normalization (1/l, hi+lo fold, channel sums, final scale) runs on the
host in fp64, which is both faster (no cross-partition folds on device —
DVE lanes are partition-locked and sub-32-partition ops at base 64
produce garbage on HW) and more accurate.

Numerics: the final x / sum(x) normalization is badly conditioned; error
sources that are correlated across queries (K and V quantization) are
amplified ~30-100x, so K and V are fed as bf16-hi + lo pairs (~22
effective bits) while Q and E ride single fp32r (~13 effective bits,
uncorrelated errors average out). All matmuls run in fp32r, which
streams at full bf16 rate (1 cycle/row for N>=256).

Per sub-problem on-device (fp32 PSUM accumulation):
  S^T[k,q] = [khi;klo].T @ [q;q]       one stacked K=128 fp32r matmul per
                                       512-col half per key chunk
  E        = exp(S^T), ACT writes fp32r directly
  PV       = [Vhi|1|Vlo].T @ E         M=128-packed: rows 0:64 hi-part,
                                       row 64 = softmax denom l,
                                       rows 65:128 lo-part
  out      = PV copied to SBUF, DMA'd raw; host folds + normalizes.
"""

import os
import numpy as np
import ml_dtypes

import concourse.bass as bass
import concourse.bacc as bacc
import concourse.mybir as mybir
import concourse.tile as tile
from concourse import bass_utils

# ---------------------------------------------------------------- constants
B, S, H, D = 2, 8192, 16, 64
SEGMENT_LENGTHS = [1024, 2048, 4096, 8192]
DILATION_RATES = [1, 2, 4, 8]
NUM_GROUPS = 4
GROUP_HEADS = H // NUM_GROUPS  # 4
SEGS_PER_GROUP = [S // s for s in SEGMENT_LENGTHS]  # [8, 4, 2, 1]
NPROB = sum(SEGS_PER_GROUP)  # 15 problems per core
SL = 1024          # per-problem sequence length (s // r, same for all groups)
NCHUNK = SL // 128  # 8 key chunks
N_CORES = 8
SCALE = 1.0 / np.sqrt(D)

FP32 = mybir.dt.float32
FP32R = mybir.dt.float32r
FP16 = mybir.dt.float16
VW = D + 1  # 65: hi rows plus the l row


def _problem_list(j):
    """15 (group, head, seg) tuples for local head-slot j, head-contiguous."""
    out = []
    for g in range(NUM_GROUPS):
        head = g * GROUP_HEADS + j
        for seg in range(SEGS_PER_GROUP[g]):
            out.append((g, head, seg))
    return out


def _positions(g, seg):
    s, r = SEGMENT_LENGTHS[g], DILATION_RATES[g]
    offset = g % r
    return seg * s + offset + r * np.arange(SL)


# ---------------------------------------------------------------- device IR
def _build_tile_program(ctx, tc, out_ap, qd_ap, kd_ap, vp_ap):
    nc = tc.nc
    EXP = mybir.ActivationFunctionType.Exp

    qk_pool = ctx.enter_context(tc.tile_pool(name="qk", bufs=3))
    k_pool = ctx.enter_context(tc.tile_pool(name="kt", bufs=3))
    vp_pool = ctx.enter_context(tc.tile_pool(name="vp", bufs=3))
    e_pool = ctx.enter_context(tc.tile_pool(name="exps", bufs=3))
    pvsb_pool = ctx.enter_context(tc.tile_pool(name="pvsb", bufs=3))
    scr_pool = ctx.enter_context(tc.tile_pool(name="scr", bufs=1))
    spsum = ctx.enter_context(tc.tile_pool(name="spsum", bufs=3, space="PSUM"))
    pvpsum = ctx.enter_context(tc.tile_pool(name="pvpsum", bufs=1, space="PSUM"))

    # preload the exp table during the initial DMA fill: a dummy
    # activation on a memset scratch tile pulls the 1.3us ACT_TABLE_LOAD
    # off the critical path
    warm = scr_pool.tile([1, 2], FP32, tag="warm")
    nc.gpsimd.memset(warm, 0.0)
    warm_o = scr_pool.tile([1, 2], FP32, tag="warmo")
    nc.scalar.activation(out=warm_o, in_=warm, func=EXP)

    reps = int(os.environ.get("DILATED_REPS", "1"))
    plist = [i % NPROB for i in range(reps * NPROB)]
    for i, p in enumerate(plist):
        last = i == len(plist) - 1
        # steady-state inputs ride the SP queue; the first problem's k/vp
        # go on the idle GpSimd queue in parallel to cut the pipeline fill
        k_t = k_pool.tile([128, SL], FP32R)
        (nc.gpsimd if i == 0 else nc.sync).dma_start(out=k_t, in_=kd_ap[p])
        qk_t = qk_pool.tile([128, SL], FP32R)
        nc.sync.dma_start(out=qk_t[0:D, :], in_=qd_ap[p])
        nc.sync.dma_start(out=qk_t[D:128, :], in_=qd_ap[p])
        vp_t = vp_pool.tile([128, NCHUNK * 128], FP16)
        (nc.gpsimd if i == 0 else nc.sync).dma_start(out=vp_t, in_=vp_ap[p])

        pv_ps = None
        for c in range(NCHUNK):
            # S^T chunk c = khi.T q + klo.T q, one stacked K=128 matmul/half
            s_ps = spsum.tile([128, SL], FP32, tag="s")
            for h in range(2):
                hs = slice(h * 512, (h + 1) * 512)
                nc.tensor.matmul(
                    out=s_ps[:, hs],
                    lhsT=k_t[:, c * 128:(c + 1) * 128],
                    rhs=qk_t[:, hs],
                    start=True, stop=True,
                )

            e_t = e_pool.tile([128, SL], FP16)
            nc.scalar.activation(out=e_t, in_=s_ps, func=EXP)

            if pv_ps is None:
                pv_ps = pvpsum.tile([128, SL], FP32, tag="pv")
            for h in range(2):
                hs = slice(h * 512, (h + 1) * 512)
                nc.tensor.matmul(      # [Vhi | 1 | Vlo].T @ E
                    out=pv_ps[:, hs],
                    lhsT=vp_t[:, c * 128:(c + 1) * 128],
                    rhs=e_t[:, hs],
                    start=(c == 0), stop=(c == NCHUNK - 1),
                )

        # evacuate PV psum to SBUF and ship it; host does the rest.
        # The last problem's output is split across two DMA queues so the
        # kernel tail is half a transfer, not a full one.
        pv_sb = pvsb_pool.tile([128, SL], FP32)
        nc.vector.tensor_copy(out=pv_sb, in_=pv_ps)
        if last:
            nc.gpsimd.dma_start(out=out_ap[p][:, 0:512], in_=pv_sb[:, 0:512])
            nc.sync.dma_start(out=out_ap[p][:, 512:SL], in_=pv_sb[:, 512:SL])
        else:
            nc.gpsimd.dma_start(out=out_ap[p], in_=pv_sb)


# Cache: the Bass program is identical for every call (and every core).
_CACHED = {}


def _get_program():
    key = os.environ.get("DILATED_REPS", "1")
    if key in _CACHED:
        return _CACHED[key]
    nc = bacc.Bacc("TRN2", target_bir_lowering=False, debug=False)
    qd = nc.dram_tensor("qd", [NPROB, D, SL], FP32R,
                        kind="ExternalInput").ap()
    kd = nc.dram_tensor("kd", [NPROB, 128, SL], FP32R,
                        kind="ExternalInput").ap()
    vp = nc.dram_tensor("vp", [NPROB, 128, NCHUNK * 128], FP16,
                        kind="ExternalInput").ap()
    out = nc.dram_tensor("out", [NPROB, 128, SL], FP32,
                         kind="ExternalOutput").ap()
    from contextlib import ExitStack
    with tile.TileContext(nc) as tc, ExitStack() as ctx:
        _build_tile_program(ctx, tc, out, qd, kd, vp)
    nc.compile()
    _CACHED[key] = nc
    return nc


# ---------------------------------------------------------------- host glue
def _bf16hi(x):
    return x.astype(ml_dtypes.bfloat16).astype(np.float32)


def _prep_core(q, k, v, b, j):
    """Build the qd/kd/vp device inputs for core (b, j). q is pre-scaled."""
    qd = np.empty((NPROB, D, SL), dtype=np.float32)
    kd = np.empty((NPROB, 128, SL), dtype=np.float32)
    vp = np.empty((NPROB, 128, NCHUNK * 128), dtype=np.float16)
    ones = np.ones((SL, 1), np.float32)
    for p, (g, head, seg) in enumerate(_problem_list(j)):
        pos = _positions(g, seg)
        qd[p] = q[b, pos, head, :].T  # [64, 1024] fp32, already scaled
        kT = k[b, pos, head, :].T
        khi = _bf16hi(kT)
        kd[p, 0:D] = khi
        kd[p, D:128] = kT - khi
        vs = v[b, pos, head, :]  # [1024, 64] fp32
        vhi = vs.astype(np.float16).astype(np.float32)
        vlo = vs - vhi
        block = np.concatenate([vhi, ones, vlo[:, 0:D - 1]], axis=1)
        vp[p] = (block.reshape(NCHUNK, 128, 128)
                 .transpose(1, 0, 2).reshape(128, NCHUNK * 128))
    return {"qd": qd, "kd": kd, "vp": vp}


def _postprocess_core(dev_out):
    """[NPROB, 128, SL] raw PV -> [NPROB, D, SL] normalized attention out.

    Folds the packed lo rows into the hi rows, divides by the softmax
    denominator row, then applies the per-(head, channel) 1/(4*sum)
    normalization across each head's segments. fp64 throughout.
    """
    pv = dev_out.astype(np.float64)
    x = pv[:, 0:D, :].copy()
    x[:, 0:D - 1, :] += pv[:, VW:128, :]
    x /= pv[:, D:VW, :]  # divide by l row
    outp = np.empty((NPROB, D, SL), dtype=np.float32)
    i = 0
    for nseg in SEGS_PER_GROUP:
        xs = x[i:i + nseg]  # [nseg, D, SL]
        hsum = xs.sum(axis=(0, 2), keepdims=True)  # [1, D, 1]
        outp[i:i + nseg] = (xs / (NUM_GROUPS * hsum)).astype(np.float32)
        i += nseg
    return outp


def kernel(query, key, value, _run_kw=None):
    q = np.asarray(query, dtype=np.float32)
    k = np.asarray(key, dtype=np.float32)
    v = np.asarray(value, dtype=np.float32)
    qs = q * SCALE  # fold softmax scale into q

    nc = _get_program()
    in_maps = []
    core_meta = []
    for core in range(N_CORES):
        b, j = divmod(core, NUM_GROUPS)
        in_maps.append(_prep_core(qs, k, v, b, j))
        core_meta.append((b, j))

    kw = dict(_run_kw or {})
    kw.pop("result", None)
    res = bass_utils.run_bass_kernel_spmd(
        nc, in_maps, core_ids=list(range(N_CORES)), **kw)

    out = np.zeros((B, S, H, D), dtype=np.float32)
    for core in range(N_CORES):
        b, j = core_meta[core]
        dev_out = _postprocess_core(res.results[core]["out"])
        for p, (g, head, seg) in enumerate(_problem_list(j)):
            pos = _positions(g, seg)
            out[b, pos, head, :] = dev_out[p].T
    if _run_kw is not None:
        _run_kw["result"] = res
    return out
